# revision 9
# baseline (speedup 1.0000x reference)
"""Bit2Num dequantization kernel for Trainium2 (8 NeuronCores, SPMD).

Reference op: x [1024, 65536] of {0.0, 1.0} f32, B=4.
  bits = x.reshape(1024, 16384, 4)
  out[b, n] = (8*bits[b,n,0] + 4*bits[b,n,1] + 2*bits[b,n,2] + bits[b,n,3] + 0.5) / 16

Sharding: pure data-parallel over batch — 128 rows per core (= 128 SBUF
partitions). Per core: 32 MB in + 8 MB out = 41.94 MB of HBM traffic.
Semaphore-crossing analysis of the DMA streams shows the sustained
per-NC HBM ceiling is ~405-410 GB/s (two NCs share one ~819 GB/s HBM3
stack), so the streaming window floor is ~102.4 us; plus ~7.8 us of
fixed framework preamble (double engine rendezvous + ucode loads) before
the first DMA descriptor can issue, plus the final load->compute->store
cascade.

Per-core schedule: pipeline over column segments:
  SWDGE (gpsimd) in-DMAs casting f32->bf16 in-flight (halves SBUF-side
  write traffic so the 435 GB/s SBUF fabric never contends) -> per chunk:
  3 fused scalar_tensor_tensor ops on DVE (u=2a+b, v=2c+d, w=4u+v over
  the 4 strided bit slices) -> final affine (w/16 + 1/32) on ACT -> DMA
  out on the ACT HWDGE ring (separate queue, stores never stall loads).
The segment list tapers at the end (1 MB -> 0.5 -> 0.25 MB) so the
compute/store cascade after the last load byte is short. Deep xin pool
(bufs=8) keeps the SWDGE queue backlogged to the end (shallow pools
starve the SDMA round-robin at the stream tail).

NOTE: do NOT put early loads on a second (HWDGE) queue: measured, the
two-queue packet round-robin settles the whole stream into a ~352 GB/s
limit cycle vs ~405 single-queue (115.8us -> 130.8us regression).
"""

import numpy as np

import concourse.bacc as bacc
import concourse.bass as bass
import concourse.mybir as mybir
from concourse.bass_utils import run_bass_kernel_spmd
from concourse.tile import TileContext

N_CORES = 8
BATCH = 1024
COLS = 65536
B_BITS = 4
ROWS = BATCH // N_CORES          # 128 rows per core == SBUF partition count
OUT_COLS = COLS // B_BITS        # 16384

F32 = mybir.dt.float32
BF16 = mybir.dt.bfloat16
MULT = mybir.AluOpType.mult
ADD = mybir.AluOpType.add

# (in_cols, [chunk_out_cols...]) per segment; all SWDGE bf16 loads.
SEGMENTS = (
    [(4096, [1024])] * 14
    + [(2048, [512])] * 4
)
assert sum(s[0] for s in SEGMENTS) == COLS
N_CHUNKS = sum(len(s[1]) for s in SEGMENTS)


def _build_nc() -> bass.Bass:
    # Bacc (not plain Bass): its compile() pipeline runs
    # generate_event_semaphores, which splits multi-wait sync conditions —
    # TRN2 DMA instructions accept at most one wait.
    nc = bacc.Bacc(None, target_bir_lowering=False)
    x = nc.dram_tensor("x", [ROWS, COLS], F32, kind="ExternalInput")
    out = nc.dram_tensor("out", [ROWS, OUT_COLS], F32, kind="ExternalOutput")

    with TileContext(nc) as tc:
        with (
            # Deep prefetch: SWDGE load N is issue-gated on DVE progress
            # N-bufs back (buffer recycle); shallow pools starve the SDMA
            # queue at the stream tail.
            tc.tile_pool(name="xin", bufs=8) as xpool,
            tc.tile_pool(name="work", bufs=4) as wpool,
            # w and ot get one buffer per chunk so they NEVER recycle:
            # recycling makes STT#3 wait on ACT progress and ACT wait on
            # store completion — each such double-wait instruction costs an
            # event semaphore (generate_event_semaphores), and every engine
            # resets every event sem serially in the teardown, which sits
            # inside the measured exec window (~0.1 us per sem per engine).
            # 15 bufs (not N_CHUNKS): SBUF is 224 KiB/partition physical but
            # the top ~16 KiB is bass-reserved — allocating into it corrupts
            # tiles (measured: NaN output at 205+ KiB of tiles). 15 bufs
            # leaves only the last 3 chunks with (never-binding) recycle
            # waits. Budget: 64(xin) + 16(uv) + 30(w) + 60(ot) + 16(DMA
            # scratch) + ~5(consts) = ~191 KiB.
            tc.tile_pool(name="wacc", bufs=15) as wxpool,
            tc.tile_pool(name="oout", bufs=15) as opool,
        ):
            col = 0
            g_off = 0
            for seg_c, chunk_gs in SEGMENTS:
                xt = xpool.tile([ROWS, seg_c], BF16, tag="xt")
                nc.gpsimd.dma_start(
                    out=xt[:, :], in_=x[:, col:col + seg_c]
                )
                col += seg_c
                c_off = 0
                for chunk_g in chunk_gs:
                    chunk_c = chunk_g * B_BITS
                    xv = xt[:, c_off:c_off + chunk_c].rearrange(
                        "p (g k) -> p g k", k=B_BITS
                    )
                    c_off += chunk_c
                    a = xv[:, :, 0]
                    b = xv[:, :, 1]
                    c = xv[:, :, 2]
                    d = xv[:, :, 3]

                    # intermediates stay bf16 (all values <= 15, exact);
                    # ACT casts back to f32 on the final affine.
                    u = wpool.tile([ROWS, chunk_g], BF16, tag="u")
                    v = wpool.tile([ROWS, chunk_g], BF16, tag="v")
                    w = wxpool.tile([ROWS, chunk_g], BF16, tag="w")
                    ot = opool.tile([ROWS, chunk_g], F32, tag="ot")

                    # u = 2a + b ; v = 2c + d ; w = 4u + v = 8a+4b+2c+d
                    nc.vector.scalar_tensor_tensor(
                        out=u[:, :], in0=a, scalar=2.0, in1=b,
                        op0=MULT, op1=ADD,
                    )
                    nc.vector.scalar_tensor_tensor(
                        out=v[:, :], in0=c, scalar=2.0, in1=d,
                        op0=MULT, op1=ADD,
                    )
                    nc.vector.scalar_tensor_tensor(
                        out=w[:, :], in0=u[:, :], scalar=4.0, in1=v[:, :],
                        op0=MULT, op1=ADD,
                    )
                    # ot = (w + 0.5) / 16 = w/16 + 1/32
                    nc.scalar.activation(
                        out=ot[:, :], in_=w[:, :],
                        func=mybir.ActivationFunctionType.Copy,
                        bias=1.0 / 32.0, scale=1.0 / 16.0,
                    )
                    # out-DMAs on the ACT HWDGE ring (qActDynamicHW) so a
                    # store waiting on compute never blocks the in-stream.
                    nc.scalar.dma_start(
                        out=out[:, g_off:g_off + chunk_g], in_=ot[:, :]
                    )
                    g_off += chunk_g
    # Bacc.finalize runs the compile pipeline (register allocation +
    # generate_event_semaphores); the pjrt exec path serializes nc.m as-is.
    nc.finalize()
    return nc


_NC = None


def _get_nc() -> bass.Bass:
    global _NC
    if _NC is None:
        _NC = _build_nc()
    return _NC


def kernel(x: np.ndarray, B=4) -> np.ndarray:
    assert int(B) == B_BITS, f"kernel hardcodes B={B_BITS}, got {B}"
    x = np.ascontiguousarray(x, dtype=np.float32)
    assert x.shape == (BATCH, COLS), x.shape
    nc = _get_nc()
    in_maps = [{"x": x[i * ROWS:(i + 1) * ROWS]} for i in range(N_CORES)]
    res = run_bass_kernel_spmd(nc, in_maps, list(range(N_CORES)))
    return np.concatenate(
        [res.results[i]["out"] for i in range(N_CORES)], axis=0
    )
